# revision 1
# baseline (speedup 1.0000x reference)
"""BART attention (B=4, S=2048, D=1024, H=16) on 8 Trainium2 NeuronCores.

Sharding: tensor-parallel across heads.  Core c owns heads {2c, 2c+1}, i.e.
projection output dims [128c, 128c+128) of wq/wk/wv and rows [128c, 128c+128)
of wo.  Each core computes its two heads' attention over the full batch and a
partial output projection; the host sums the 8 partial outputs.

Device layout per core (all matmuls in float32r: full PE speed, ~1e-4 rel):
  qT, kT  [128 head-dims, 8192 tokens]   (transposed projections)
  v_comb  [tokens, 130] = [vA(64) | 1 | vB(64) | 1]  (ones col -> softmax sums)
  scoresT [k-tok, q-tok] per (batch, head): softmax denom = extra out row of
  the ones-augmented attn@v matmul; exp on ScalarE with fused 1/8 scale; the
  1/sum normalization is applied after attn@v (flash-attention style).
"""
import numpy as np

import concourse.bass as bass
import concourse.mybir as mybir
import concourse.tile as tile
from concourse.bass_utils import run_bass_kernel_spmd
from concourse.masks import make_identity
from concourse.vector_clock import ScopedClock

F32 = mybir.dt.float32
F32R = mybir.dt.float32r
EXPF = mybir.ActivationFunctionType.Exp

B, S, D = 4, 2048, 1024
T = B * S                      # 8192 tokens
NCORES = 8
P = 128                        # partitions / head-dims per core
DK = 64                        # head dim
KC = D // P                    # 8 contraction chunks for projections
TCH = 512                      # token chunk (projection N / q-chunk)
NTCH = T // TCH                # 16
VW = 2 * DK + 2                # 130: [vA | 1 | vB | 1]

# ---------------------------------------------------------------------------
# walrus in this toolchain encodes at most ONE sync wait per instruction
# (two on EventSemaphore).  Tile emits more.  Legalize by carrying excess
# waits on same-engine NOPs inserted right before the instruction (engines
# execute in order, so this is equivalent), and by splitting the kernel-tail
# drain's global-clock waits across a chain of drains.
# ---------------------------------------------------------------------------
_split_counter = [0]


def _legalize_waits(nc):
    inserted = 0
    for fn in nc.m.functions:
        for bb in fn.blocks:
            new_insts = []
            changed = False
            for inst in bb.instructions:
                si = inst.sync_info
                waits = list(si.on_wait) if si is not None and si.on_wait else []
                cap = 2 if inst.opcode == "EventSemaphore" else 1
                if len(waits) > cap:
                    excess, keep = waits[:-cap], waits[-cap:]
                    for w in excess:
                        _split_counter[0] += 1
                        nop = mybir.InstNoOp(
                            name=f"I-waitsplit-{_split_counter[0]}", ins=[], outs=[]
                        )
                        nop.engine = inst.engine
                        nop.sync_info = mybir.SyncInfo(on_wait=[w], on_update=[])
                        new_insts.append(nop)
                        inserted += 1
                    si.on_wait = keep
                    changed = True
                new_insts.append(inst)
            if changed:
                bb.instructions.clear()
                for i in new_insts:
                    bb.instructions.append(i)
    return inserted


class _TC(tile.TileContext):
    def _drain_and_barrier(self, tick_clock, wait_clock):
        drain_inst = self.nc.sync.drain()
        wait_clock.add_sem_waits(
            drain_inst.ins, ScopedClock({None: tick_clock.global_clock})
        )
        si = drain_inst.ins.sync_info
        waits = list(si.on_wait or []) if si is not None else []
        if len(waits) > 1:
            si.on_wait = [waits[0]]
            for w in waits[1:]:
                d = self.nc.sync.drain()
                dsi = d.ins.sync_info
                if dsi is None:
                    d.ins.sync_info = mybir.SyncInfo(on_wait=[w], on_update=[])
                else:
                    dsi.on_wait = [w]
        self.nc.all_engine_barrier()
        assert self.sems is not None
        popped = self.nc._tile_sem_poison_stack.pop()
        assert popped is self._sem_poison
        self.nc.clear_and_free_semaphores(list(self.sems.allocated().values()))
        self.nc.all_engine_barrier()


# ---------------------------------------------------------------------------
# device program (identical on all 8 cores; only input data differs)
# ---------------------------------------------------------------------------
def _build_nc(repeat=1):
    nc = bass.Bass("TRN2", target_bir_lowering=False, debug=False,
                   num_devices=NCORES)
    xt = nc.dram_tensor("xt", [D, T], F32R, kind="ExternalInput").ap()
    wqm = nc.dram_tensor("wqm", [D, P], F32R, kind="ExternalInput").ap()
    wqb = nc.dram_tensor("wqb", [1, P], F32R, kind="ExternalInput").ap()
    wkm = nc.dram_tensor("wkm", [D, P], F32R, kind="ExternalInput").ap()
    wkb = nc.dram_tensor("wkb", [1, P], F32R, kind="ExternalInput").ap()
    wvm = nc.dram_tensor("wvm", [D, P], F32R, kind="ExternalInput").ap()
    wvb = nc.dram_tensor("wvb", [1, P], F32R, kind="ExternalInput").ap()
    wot = nc.dram_tensor("wo", [P, D], F32R, kind="ExternalInput").ap()
    bot = nc.dram_tensor("bo", [KC, P], F32, kind="ExternalInput").ap()
    yt = nc.dram_tensor("yt", [D, T], F32, kind="ExternalOutput").ap()

    with _TC(nc) as tc, nc.allow_low_precision(
            reason="float32r is 32-bit; PE rounds internally"):
        _emit(nc, tc, xt, wqm, wqb, wkm, wkb, wvm, wvb, wot, bot, yt,
              repeat=repeat)
    n = _legalize_waits(nc)
    return nc, n


def _emit(nc, tc, xt, wqm, wqb, wkm, wkb, wvm, wvb, wot, bot, yt, repeat=1):
    ctxs = []

    def pool(name, bufs, space="SBUF"):
        p = tc.tile_pool(name=name, bufs=bufs, space=space)
        ctxs.append(p)
        return p.__enter__()

    wpool = pool("w", 1)
    persist = pool("persist", 1)
    xpool = pool("x", 2)
    scrpool = pool("scr", 2)
    epool = pool("e", 3)
    orawpool = pool("oraw", 2)
    sumpool = pool("sums", 2)
    stgpool = pool("stg", 2)
    ystpool = pool("yst", 2)
    spool = pool("ps_s", 2, space="PSUM")     # [128,1024] = 2 banks/slot
    opool = pool("ps_o", 2, space="PSUM")     # 1 bank/slot
    ypool = pool("ps_y", 2, space="PSUM")     # 1 bank/slot

    # ---- constants / weights (loaded once) ----
    wq_sb = wpool.tile([P, KC, P], F32R)
    wk_sb = wpool.tile([P, KC, P], F32R)
    wv_sb = wpool.tile([P, KC, P], F32R)
    nc.sync.dma_start(wq_sb[:], wqm.rearrange("(k p) d -> p k d", p=P))
    nc.sync.dma_start(wk_sb[:], wkm.rearrange("(k p) d -> p k d", p=P))
    nc.sync.dma_start(wv_sb[:], wvm.rearrange("(k p) d -> p k d", p=P))
    wqb_sb = wpool.tile([1, P], F32R)
    wkb_sb = wpool.tile([1, P], F32R)
    wvb_sb = wpool.tile([1, P], F32R)
    nc.sync.dma_start(wqb_sb[:], wqb[0:1, :])
    nc.sync.dma_start(wkb_sb[:], wkb[0:1, :])
    nc.sync.dma_start(wvb_sb[:], wvb[0:1, :])
    wo_sb = wpool.tile([P, D], F32R)
    nc.sync.dma_start(wo_sb[:], wot[:, :])
    bo_sb = wpool.tile([P, KC], F32)
    nc.sync.dma_start(bo_sb[:], bot.rearrange("m p -> p m"))
    # memset can't write float32r; memset f32 then DVE-copy (which rounds)
    ones_f32 = wpool.tile([P, TCH], F32)
    nc.vector.memset(ones_f32[:], 1.0)
    ones_sb = wpool.tile([1, TCH], F32R)
    nc.vector.tensor_copy(ones_sb[:], ones_f32[0:1, :])
    ident = wpool.tile([P, P], F32)
    make_identity(nc, ident[:])

    # ---- persistent activations ----
    qT = persist.tile([P, T], F32R)
    kT = persist.tile([P, T], F32R)
    v_comb = persist.tile([P, T // P, VW], F32R)    # [tok%128, tok-tile, 130]
    nc.vector.tensor_copy(
        v_comb[:, :, DK:DK + 1],
        ones_f32[:, 0:1].broadcast_to([P, T // P, 1]))
    nc.vector.tensor_copy(
        v_comb[:, :, VW - 1:VW],
        ones_f32[:, 0:1].broadcast_to([P, T // P, 1]))

    NQC = S // TCH                # 4 q-chunks per batch
    NKT = S // P                  # 16 k-tiles per batch

    for b in [b for _ in range(repeat) for b in range(B)]:
        t0 = b * S
        # ================= phase P: q/k/v projections for batch b ==========
        for i in range(S // TCH):
            c0 = t0 + i * TCH
            x_ch = xpool.tile([P, KC, TCH], F32R, tag="x")
            nc.sync.dma_start(
                x_ch[:], xt[:, c0:c0 + TCH].rearrange("(k p) n -> p k n", p=P))
            s_t = spool.tile([P, 2 * TCH], F32, tag="s")
            v_ps = ypool.tile([P, TCH], F32, tag="y")
            for kc in range(KC):
                st = kc == 0
                nc.tensor.matmul(s_t[:, 0:TCH], wq_sb[:, kc, :], x_ch[:, kc, :],
                                 start=st, stop=False)
                nc.tensor.matmul(s_t[:, TCH:2 * TCH], wk_sb[:, kc, :],
                                 x_ch[:, kc, :], start=st, stop=False)
                nc.tensor.matmul(v_ps[:], wv_sb[:, kc, :], x_ch[:, kc, :],
                                 start=st, stop=False)
            nc.tensor.matmul(s_t[:, 0:TCH], wqb_sb[:], ones_sb[:],
                             start=False, stop=True)
            nc.tensor.matmul(s_t[:, TCH:2 * TCH], wkb_sb[:], ones_sb[:],
                             start=False, stop=True)
            nc.tensor.matmul(v_ps[:], wvb_sb[:], ones_sb[:],
                             start=False, stop=True)
            nc.vector.tensor_copy(qT[:, c0:c0 + TCH], s_t[:, 0:TCH])
            nc.vector.tensor_copy(kT[:, c0:c0 + TCH], s_t[:, TCH:2 * TCH])
            v_scr = scrpool.tile([P, TCH], F32, tag="vscr")
            nc.vector.tensor_copy(v_scr[:], v_ps[:])
            for tt in range(TCH // P):
                vt = (c0 // P) + tt
                tr = opool.tile([P, TCH], F32, tag="o")
                nc.tensor.transpose(tr[:, 0:P], v_scr[:, tt * P:(tt + 1) * P],
                                    ident[:])
                nc.vector.tensor_copy(v_comb[:, vt, 0:DK], tr[:, 0:DK])
                nc.vector.tensor_copy(v_comb[:, vt, DK + 1:2 * DK + 1],
                                      tr[:, DK:2 * DK])

        # ================= phase A: attention for batch b ==================
        sums_pp = sumpool.tile([2 * NQC, TCH], F32, tag="sumpp")
        oraw = orawpool.tile([P, S], F32R, tag="oraw")
        for qc in range(NQC):
            q0 = t0 + qc * TCH
            ps_oA = opool.tile([DK + 1, TCH], F32, tag="o")
            ps_oB = opool.tile([DK + 1, TCH], F32, tag="o")
            # software pipeline: attn@v for kc runs one step behind the
            # scores/exp of kc+1 so the PE never serializes behind ACT.
            def attnv(kc, e_t):
                vt = (t0 // P) + kc
                nc.tensor.matmul(ps_oA[:], v_comb[:, vt, 0:DK + 1],
                                 e_t[:, 0:TCH],
                                 start=(kc == 0), stop=(kc == NKT - 1))
                nc.tensor.matmul(ps_oB[:], v_comb[:, vt, DK + 1:VW],
                                 e_t[:, TCH:2 * TCH],
                                 start=(kc == 0), stop=(kc == NKT - 1))

            pending = None
            for kc in range(NKT):
                kt0 = t0 + kc * P
                s_t = spool.tile([P, 2 * TCH], F32, tag="s")
                nc.tensor.matmul(s_t[:, 0:TCH], kT[0:DK, kt0:kt0 + P],
                                 qT[0:DK, q0:q0 + TCH], start=True, stop=True)
                nc.tensor.matmul(s_t[:, TCH:2 * TCH], kT[DK:P, kt0:kt0 + P],
                                 qT[DK:P, q0:q0 + TCH], start=True, stop=True)
                e_t = epool.tile([P, 2 * TCH], F32R, tag="e")
                nc.scalar.activation(e_t[:], s_t[:], EXPF, scale=0.125)
                if pending is not None:
                    attnv(*pending)
                pending = (kc, e_t)
            attnv(*pending)
            # stash softmax denominators (row DK) and raw outputs.
            # DVE writes must start at a 32-aligned partition, so stage each
            # sums row at partition 0 and DMA it to its sums_pp row.
            s_stgA = stgpool.tile([1, TCH], F32, tag="sstg")
            nc.vector.tensor_copy(s_stgA[:], ps_oA[DK:DK + 1, :])
            nc.sync.dma_start(sums_pp[2 * qc:2 * qc + 1, :], s_stgA[:])
            s_stgB = stgpool.tile([1, TCH], F32, tag="sstg")
            nc.vector.tensor_copy(s_stgB[:], ps_oB[DK:DK + 1, :])
            nc.sync.dma_start(sums_pp[2 * qc + 1:2 * qc + 2, :], s_stgB[:])
            nc.vector.tensor_copy(oraw[0:DK, qc * TCH:(qc + 1) * TCH],
                                  ps_oA[0:DK, :])
            nc.vector.tensor_copy(oraw[DK:P, qc * TCH:(qc + 1) * TCH],
                                  ps_oB[0:DK, :])
        # normalization: r = 1/sums, broadcast over 64 partitions, multiply
        recip_pp = sumpool.tile([2 * NQC, TCH], F32R, tag="recip")
        nc.vector.reciprocal(recip_pp[:], sums_pp[:])
        for qc in range(NQC):
            for h in range(2):
                r = 2 * qc + h
                stg = stgpool.tile([1, TCH], F32R, tag="stg")
                nc.sync.dma_start(stg[:], recip_pp[r:r + 1, :])
                bc = spool.tile([P, 2 * TCH], F32, tag="s")
                nc.tensor.matmul(bc[0:DK, 0:TCH], ones_sb[0:1, 0:DK], stg[:],
                                 start=True, stop=True)
                sl = slice(qc * TCH, (qc + 1) * TCH)
                nc.vector.tensor_mul(oraw[h * DK:(h + 1) * DK, sl],
                                     oraw[h * DK:(h + 1) * DK, sl],
                                     bc[0:DK, 0:TCH])

        # ================= phase O: output projection for batch b ==========
        for m in range(KC):
            for qc in range(NQC):
                ps_y = ypool.tile([P, TCH], F32, tag="y")
                nc.tensor.matmul(ps_y[:], wo_sb[:, m * P:(m + 1) * P],
                                 oraw[:, qc * TCH:(qc + 1) * TCH],
                                 start=True, stop=True)
                ys = ystpool.tile([P, TCH], F32, tag="yst")
                nc.vector.tensor_scalar_add(ys[:], ps_y[:], bo_sb[:, m:m + 1])
                nc.sync.dma_start(
                    yt[m * P:(m + 1) * P, t0 + qc * TCH:t0 + (qc + 1) * TCH],
                    ys[:])

    for p in reversed(ctxs):
        p.__exit__(None, None, None)


_CACHED = {}


def _get_nc(repeat=1):
    if repeat not in _CACHED:
        _CACHED[repeat] = _build_nc(repeat=repeat)[0]
    return _CACHED[repeat]


def _make_in_maps(x, wq, bq, wk, bk, wv, bv, wo, bo):
    x = np.asarray(x, np.float32)
    wq, bq = np.asarray(wq, np.float32), np.asarray(bq, np.float32)
    wk, bk = np.asarray(wk, np.float32), np.asarray(bk, np.float32)
    wv, bv = np.asarray(wv, np.float32), np.asarray(bv, np.float32)
    wo, bo = np.asarray(wo, np.float32), np.asarray(bo, np.float32)
    xT = np.ascontiguousarray(x.reshape(T, D).T)
    maps = []
    for c in range(NCORES):
        sl = slice(c * P, (c + 1) * P)
        maps.append({
            "xt": xT,
            "wqm": np.ascontiguousarray(wq[:, sl]),
            "wqb": np.ascontiguousarray(bq[sl])[None, :],
            "wkm": np.ascontiguousarray(wk[:, sl]),
            "wkb": np.ascontiguousarray(bk[sl])[None, :],
            "wvm": np.ascontiguousarray(wv[:, sl]),
            "wvb": np.ascontiguousarray(bv[sl])[None, :],
            "wo": np.ascontiguousarray(wo[sl, :]),
            "bo": (bo if c == 0 else np.zeros_like(bo)).reshape(KC, P).copy(),
        })
    return maps


def kernel(x, wq, bq, wk, bk, wv, bv, wo, bo):
    nc = _get_nc()
    in_maps = _make_in_maps(x, wq, bq, wk, bk, wv, bv, wo, bo)
    res = run_bass_kernel_spmd(nc, in_maps, core_ids=list(range(NCORES)),
                               trace=False)
    yT = res.results[0]["yt"].copy()
    for c in range(1, NCORES):
        yT += res.results[c]["yt"]
    return np.ascontiguousarray(yT.T).reshape(B, S, D)



# revision 2
# speedup vs baseline: 3.1531x; 3.1531x over previous
"""BART attention (B=4, S=2048, D=1024, H=16) on 8 Trainium2 NeuronCores.

Sharding: DP4 x TP2.  Core c owns batch c//2 and head half c%2 (8 heads =
512 projection dims), processed as 4 head-pair "slices" of 128 dims each.
Host sums the two partial y's per batch and adds bo.

Per-core schedule (all matmul inputs bf16; PSUM accumulates f32):
  - x for the core's batch is DMA'd once and stays resident in SBUF.
  - slice s+1's q/k/v projections are interleaved into slice s's attention
    so the PE never stalls behind the softmax-exp stream on ScalarE.
  - v is computed directly in [token, head-dim] orientation (no transposes);
    q/k biases ride the PSUM->SBUF drain (DVE tensor_scalar_add); v bias is
    a rank-1 ones x bv matmul into the same PSUM tile.
  - softmax: exp on ScalarE (fused 1/8 scale); denominators come free as
    extra ones-columns in v_comb -> one PSUM row each; per-q-chunk
    normalization: copy the two sums rows to partition 0, one DVE
    reciprocal, two rank-1 broadcast matmuls, one DVE multiply.  No DMA
    round trips.
  - out-proj runs at the tail (contraction over all 4 slices accumulating
    in PSUM), overlapped with the last slice's attention; y chunks DMA
    straight from PSUM.
"""
import numpy as np
import ml_dtypes

import concourse.bass as bass
import concourse.mybir as mybir
import concourse.tile as tile
from concourse.bass_utils import run_bass_kernel_spmd
from concourse.vector_clock import ScopedClock

F32 = mybir.dt.float32
F32R = mybir.dt.float32r
BF16 = mybir.dt.bfloat16
EXPF = mybir.ActivationFunctionType.Exp

B, S, D = 4, 2048, 1024
NCORES = 8
P = 128                        # partitions / head-dims per slice
DK = 64                        # head dim
KC = D // P                    # 8 contraction chunks for projections
TCH = 512                      # token chunk (projection N / q-chunk)
NCH = S // TCH                 # 4 token chunks per batch
NSL = 4                        # head-pair slices per core (4*128 = 512 dims)
NKT = S // P                   # 16 k-tiles per q-chunk
VW = 2 * DK + 2                # 130: [1 | vA | vB | 1]
NM = D // P                    # 8 output-dim chunks of out-proj

# ---------------------------------------------------------------------------
# walrus in this toolchain encodes at most ONE sync wait per instruction
# (two on EventSemaphore).  Tile emits more.  Legalize by carrying excess
# waits on same-engine NOPs inserted right before the instruction (engines
# execute in order, so this is equivalent), and by splitting the kernel-tail
# drain's global-clock waits across a chain of drains.
# ---------------------------------------------------------------------------
_split_counter = [0]


def _legalize_waits(nc):
    inserted = 0
    for fn in nc.m.functions:
        for bb in fn.blocks:
            new_insts = []
            changed = False
            for inst in bb.instructions:
                si = inst.sync_info
                waits = list(si.on_wait) if si is not None and si.on_wait else []
                cap = 2 if inst.opcode == "EventSemaphore" else 1
                if len(waits) > cap:
                    excess, keep = waits[:-cap], waits[-cap:]
                    for w in excess:
                        _split_counter[0] += 1
                        nop = mybir.InstNoOp(
                            name=f"I-waitsplit-{_split_counter[0]}", ins=[], outs=[]
                        )
                        nop.engine = inst.engine
                        nop.sync_info = mybir.SyncInfo(on_wait=[w], on_update=[])
                        new_insts.append(nop)
                        inserted += 1
                    si.on_wait = keep
                    changed = True
                new_insts.append(inst)
            if changed:
                bb.instructions.clear()
                for i in new_insts:
                    bb.instructions.append(i)
    return inserted


class _TC(tile.TileContext):
    def _drain_and_barrier(self, tick_clock, wait_clock):
        drain_inst = self.nc.sync.drain()
        wait_clock.add_sem_waits(
            drain_inst.ins, ScopedClock({None: tick_clock.global_clock})
        )
        si = drain_inst.ins.sync_info
        waits = list(si.on_wait or []) if si is not None else []
        if len(waits) > 1:
            si.on_wait = [waits[0]]
            for w in waits[1:]:
                d = self.nc.sync.drain()
                dsi = d.ins.sync_info
                if dsi is None:
                    d.ins.sync_info = mybir.SyncInfo(on_wait=[w], on_update=[])
                else:
                    dsi.on_wait = [w]
        self.nc.all_engine_barrier()
        assert self.sems is not None
        popped = self.nc._tile_sem_poison_stack.pop()
        assert popped is self._sem_poison
        self.nc.clear_and_free_semaphores(list(self.sems.allocated().values()))
        self.nc.all_engine_barrier()


# ---------------------------------------------------------------------------
# device program (identical on all 8 cores; only input data differs)
# ---------------------------------------------------------------------------
def _build_nc(repeat=1):
    nc = bass.Bass("TRN2", target_bir_lowering=False, debug=False,
                   num_devices=NCORES)
    xt = nc.dram_tensor("xt", [D, S], BF16, kind="ExternalInput").ap()
    wqm = nc.dram_tensor("wqm", [D, NSL * P], BF16, kind="ExternalInput").ap()
    wkm = nc.dram_tensor("wkm", [D, NSL * P], BF16, kind="ExternalInput").ap()
    wvm = nc.dram_tensor("wvm", [D, NSL * P], BF16, kind="ExternalInput").ap()
    wqb = nc.dram_tensor("wqb", [NSL, P], F32, kind="ExternalInput").ap()
    wkb = nc.dram_tensor("wkb", [NSL, P], F32, kind="ExternalInput").ap()
    wvb = nc.dram_tensor("wvb", [1, NSL * P], BF16, kind="ExternalInput").ap()
    wot = nc.dram_tensor("wo", [NSL * P, D], BF16, kind="ExternalInput").ap()
    yt = nc.dram_tensor("yt", [D, S], BF16, kind="ExternalOutput").ap()

    with _TC(nc) as tc, nc.allow_low_precision(
            reason="bf16 matmul inputs; 2e-2 harness tolerance"):
        _emit(nc, tc, xt, wqm, wkm, wvm, wqb, wkb, wvb, wot, yt, repeat=repeat)
    _legalize_waits(nc)
    return nc


def _emit(nc, tc, xt, wqm, wkm, wvm, wqb, wkb, wvb, wot, yt, repeat=1):
    ctxs = []

    def pool(name, bufs, space="SBUF"):
        p = tc.tile_pool(name=name, bufs=bufs, space=space)
        ctxs.append(p)
        return p.__enter__()

    wpool = pool("w", 1)
    persist = pool("persist", 1)
    qkpool = pool("qk", 2)
    vpool = pool("v", 2)
    epool = pool("e", 3)
    sumpool = pool("sums", 2)
    ypool = pool("yst", 2)
    spool = pool("ps_s", 2, space="PSUM")      # [128,1024] f32 = 2 banks/slot
    opool = pool("ps_o", 2, space="PSUM")      # [65,512] 1 bank/slot (A+B)
    gpool = pool("ps_g", 2, space="PSUM")      # [128,512] 1 bank/slot (shared)

    # ---- constants / weights / resident x ----
    # DMA queue order is issue order: x chunk 0 and wq first so the first
    # projection matmuls start as early as possible.
    x_res = persist.tile([P, KC, S], BF16)      # resident x [d%128, d//128, t]
    wq_sb = wpool.tile([P, KC, NSL * P], BF16)
    wk_sb = wpool.tile([P, KC, NSL * P], BF16)
    wv_sb = wpool.tile([P, KC, NSL * P], BF16)
    wo_sb = wpool.tile([P, NSL, D], BF16)
    bq_sb = wpool.tile([P, NSL], F32)
    bk_sb = wpool.tile([P, NSL], F32)
    bv_sb = wpool.tile([1, NSL * P], BF16)

    def xload(c0):
        nc.sync.dma_start(
            x_res[:, :, c0:c0 + TCH],
            xt[:, c0:c0 + TCH].rearrange("(k p) n -> p k n", p=P))

    xload(0)
    nc.sync.dma_start(wq_sb[:], wqm.rearrange("(k p) d -> p k d", p=P))
    nc.sync.dma_start(bq_sb[:], wqb.rearrange("s p -> p s"))
    nc.sync.dma_start(wk_sb[:], wkm.rearrange("(k p) d -> p k d", p=P))
    nc.sync.dma_start(bk_sb[:], wkb.rearrange("s p -> p s"))
    nc.sync.dma_start(wv_sb[:], wvm.rearrange("(k p) d -> p k d", p=P))
    nc.sync.dma_start(bv_sb[:], wvb[0:1, :])
    for c in range(1, NCH):
        xload(c * TCH)
    nc.sync.dma_start(wo_sb[:], wot.rearrange("(s p) d -> p s d", p=P))

    ones_f32 = wpool.tile([P, TCH], F32)
    nc.vector.memset(ones_f32[:], 1.0)
    ones_bf = wpool.tile([1, TCH], BF16)
    nc.vector.tensor_copy(ones_bf[:], ones_f32[0:1, :])
    ones_r = wpool.tile([1, DK], F32R)
    nc.vector.tensor_copy(ones_r[:], ones_f32[0:1, 0:DK])

    # all slices' normalized attention outputs: [dim%128, slice, tok]
    oraw = persist.tile([P, NSL, S], BF16)

    def alloc_slice_tiles():
        qT = qkpool.tile([P, S], BF16, tag="qT")
        kT = qkpool.tile([P, S], BF16, tag="kT")
        v_comb = vpool.tile([P, NKT, VW], BF16, tag="v")
        nc.vector.tensor_copy(
            v_comb[:, :, DK:DK + 1],
            ones_f32[:, 0:1].broadcast_to([P, NKT, 1]))
        nc.vector.tensor_copy(
            v_comb[:, :, VW - 1:VW],
            ones_f32[:, 0:1].broadcast_to([P, NKT, 1]))
        return qT, kT, v_comb

    # ---------------- projection steps for slice s (generator) -------------
    def proj_steps(s, tiles):
        """Yields closures; each emits a small group of instructions that
        computes slice s's qT/kT/v_comb into `tiles`."""
        qT, kT, v_comb = tiles
        ps = [None]
        for c in range(NCH):
            c0 = c * TCH

            def qkmm(c0, w_sb, lo):
                if lo == 0:
                    ps[0] = gpool.tile([P, TCH], F32, tag="g", name="qk_ps")
                for kc in range(lo, lo + 4):
                    nc.tensor.matmul(ps[0][:], w_sb[:, kc, s * P:(s + 1) * P],
                                     x_res[:, kc, c0:c0 + TCH],
                                     start=(kc == 0), stop=(kc == KC - 1))

            def qkdrain(c0, dst, b_sb):
                nc.vector.tensor_scalar_add(dst[:, c0:c0 + TCH], ps[0][:],
                                            b_sb[:, s:s + 1])

            yield lambda c0=c0: qkmm(c0, wq_sb, 0)
            yield lambda c0=c0: qkmm(c0, wq_sb, 4)
            yield lambda c0=c0: qkdrain(c0, qT, bq_sb)
            yield lambda c0=c0: qkmm(c0, wk_sb, 0)
            yield lambda c0=c0: qkmm(c0, wk_sb, 4)
            yield lambda c0=c0: qkdrain(c0, kT, bk_sb)

            # v in [token, head-dim] orientation: 4 token-tiles of 128
            def vmm(c0, pair):
                if pair == 0:
                    ps[0] = gpool.tile([P, TCH], F32, tag="g", name="v_ps")
                for tt in range(2 * pair, 2 * pair + 2):
                    t0 = c0 + tt * P
                    for kc in range(KC):
                        nc.tensor.matmul(ps[0][:, tt * P:(tt + 1) * P],
                                         x_res[:, kc, t0:t0 + P],
                                         wv_sb[:, kc, s * P:(s + 1) * P],
                                         start=(kc == 0), stop=False)
                    nc.tensor.matmul(ps[0][:, tt * P:(tt + 1) * P],
                                     ones_bf[0:1, 0:P],
                                     bv_sb[0:1, s * P:(s + 1) * P],
                                     start=False, stop=True)

            def vdrain(c0, half):
                vt0 = c0 // P
                o = half * (DK + 1)
                nc.vector.tensor_copy(
                    v_comb[:, vt0:vt0 + 4, o:o + DK],
                    ps[0].rearrange("p (t d) -> p t d", t=4)[:, :,
                                                            half * DK:
                                                            (half + 1) * DK])

            yield lambda c0=c0: vmm(c0, 0)
            yield lambda c0=c0: vmm(c0, 1)
            yield lambda c0=c0: vdrain(c0, 0)
            yield lambda c0=c0: vdrain(c0, 1)

    # ---------------- output projection for q-chunk qc ---------------------
    def outproj(qc):
        q0 = qc * TCH
        for m in range(NM):
            ps_y = gpool.tile([P, TCH], F32, tag="g")
            for s in range(NSL):
                nc.tensor.matmul(ps_y[:], wo_sb[:, s, m * P:(m + 1) * P],
                                 oraw[:, s, q0:q0 + TCH],
                                 start=(s == 0), stop=(s == NSL - 1))
            ys = ypool.tile([P, TCH], BF16, tag="ys")
            nc.vector.tensor_copy(ys[:], ps_y[:])
            nc.sync.dma_start(yt[m * P:(m + 1) * P, q0:q0 + TCH], ys[:])

    # ---------------- attention for slice s, interleaved -------------------
    def attention(s, tiles, inter, outproj_here):
        qT, kT, v_comb = tiles
        pend = []
        for qc in range(NCH):
            q0 = qc * TCH
            ps_oA = opool.tile([DK + 1, TCH], F32, tag="o")
            ps_oB = opool.tile([DK + 1, TCH], F32, tag="o")

            def attnv(kc, e_t):
                nc.tensor.matmul(ps_oA[:], v_comb[:, kc, 0:DK + 1],
                                 e_t[:, 0:TCH],
                                 start=(kc == 0), stop=(kc == NKT - 1))
                nc.tensor.matmul(ps_oB[:], v_comb[:, kc, DK + 1:VW],
                                 e_t[:, TCH:2 * TCH],
                                 start=(kc == 0), stop=(kc == NKT - 1))

            for kc in range(NKT):
                kt0 = kc * P
                s_t = spool.tile([P, 2 * TCH], F32, tag="s")
                nc.tensor.matmul(s_t[:, 0:TCH], kT[0:DK, kt0:kt0 + P],
                                 qT[0:DK, q0:q0 + TCH], start=True, stop=True)
                nc.tensor.matmul(s_t[:, TCH:2 * TCH], kT[DK:P, kt0:kt0 + P],
                                 qT[DK:P, q0:q0 + TCH], start=True, stop=True)
                e_t = epool.tile([P, 2 * TCH], BF16, tag="e")
                nc.scalar.activation(e_t[:], s_t[:], EXPF, scale=0.125)
                if len(pend) >= 2:
                    attnv(*pend.pop(0))
                pend.append((kc, e_t))
                if inter:
                    step = next(inter[0], None)
                    if step is None:
                        inter.pop(0)
                    else:
                        step()
            while pend:
                attnv(*pend.pop(0))

            # ---- normalization for q-chunk qc (no DMA round trips) ----
            # v_comb column order per k-tile is [vA | 1 | vB | 1]: both
            # softmax sums land on partition 64 (32-aligned, as DVE PSUM
            # access requires), v outs on partitions 0:64.
            sums_t = sumpool.tile([1, 2 * TCH], F32, tag="sums")
            nc.vector.tensor_copy(sums_t[:, 0:TCH], ps_oA[DK:DK + 1, :])
            nc.vector.tensor_copy(sums_t[:, TCH:2 * TCH], ps_oB[DK:DK + 1, :])
            nc.vector.tensor_copy(oraw[0:DK, s, q0:q0 + TCH], ps_oA[0:DK, :])
            nc.vector.tensor_copy(oraw[DK:P, s, q0:q0 + TCH], ps_oB[0:DK, :])
            recip_t = sumpool.tile([1, 2 * TCH], F32R, tag="recip")
            nc.vector.reciprocal(recip_t[:], sums_t[:])
            bcA = gpool.tile([P, TCH], F32, tag="g", name="bcA")
            nc.tensor.matmul(bcA[0:DK, :], ones_r[:], recip_t[:, 0:TCH],
                             start=True, stop=True)
            bcB = gpool.tile([P, TCH], F32, tag="g", name="bcB")
            nc.tensor.matmul(bcB[0:DK, :], ones_r[:], recip_t[:, TCH:2 * TCH],
                             start=True, stop=True)
            nc.vector.tensor_mul(oraw[0:DK, s, q0:q0 + TCH],
                                 oraw[0:DK, s, q0:q0 + TCH], bcA[0:DK, :])
            nc.vector.tensor_mul(oraw[DK:P, s, q0:q0 + TCH],
                                 oraw[DK:P, s, q0:q0 + TCH], bcB[0:DK, :])
            if outproj_here:
                outproj(qc)

    # ---------------- top-level schedule -----------------------------------
    total = NSL * repeat
    cur = alloc_slice_tiles()
    for st in proj_steps(0, cur):
        st()
    for i in range(total):
        s = i % NSL
        inter = []
        nxt = None
        if i + 1 < total:
            nxt = alloc_slice_tiles()
            inter = [proj_steps((i + 1) % NSL, nxt)]
        attention(s, cur, inter, outproj_here=(s == NSL - 1))
        cur = nxt

    for p in reversed(ctxs):
        p.__exit__(None, None, None)


_CACHED = {}


def _get_nc(repeat=1):
    if repeat not in _CACHED:
        _CACHED[repeat] = _build_nc(repeat=repeat)
    return _CACHED[repeat]


def _make_in_maps(x, wq, bq, wk, bk, wv, bv, wo, bo):
    x = np.asarray(x, np.float32)
    wq, bq = np.asarray(wq, np.float32), np.asarray(bq, np.float32)
    wk, bk = np.asarray(wk, np.float32), np.asarray(bk, np.float32)
    wv, bv = np.asarray(wv, np.float32), np.asarray(bv, np.float32)
    wo = np.asarray(wo, np.float32)
    bf = ml_dtypes.bfloat16
    maps = []
    for c in range(NCORES):
        b, h = c // 2, c % 2
        sl = slice(h * NSL * P, (h + 1) * NSL * P)
        maps.append({
            "xt": np.ascontiguousarray(x[b].T).astype(bf),
            "wqm": np.ascontiguousarray(wq[:, sl]).astype(bf),
            "wkm": np.ascontiguousarray(wk[:, sl]).astype(bf),
            "wvm": np.ascontiguousarray(wv[:, sl]).astype(bf),
            "wqb": np.ascontiguousarray(bq[sl]).reshape(NSL, P),
            "wkb": np.ascontiguousarray(bk[sl]).reshape(NSL, P),
            "wvb": np.ascontiguousarray(bv[sl]).reshape(1, NSL * P).astype(bf),
            "wo": np.ascontiguousarray(wo[sl, :]).astype(bf),
        })
    return maps


def _gather(results, bo):
    """results: list of 8 dicts with 'yt' [D, S] partial sums."""
    bo = np.asarray(bo, np.float32)
    y = np.empty((B, S, D), np.float32)
    for b in range(B):
        yT = results[2 * b]["yt"].astype(np.float32) + \
            results[2 * b + 1]["yt"].astype(np.float32)
        y[b] = yT.T + bo
    return y


def kernel(x, wq, bq, wk, bk, wv, bv, wo, bo):
    nc = _get_nc()
    in_maps = _make_in_maps(x, wq, bq, wk, bk, wv, bv, wo, bo)
    res = run_bass_kernel_spmd(nc, in_maps, core_ids=list(range(NCORES)),
                               trace=False)
    return _gather(res.results, bo)
